# revision 1
# baseline (speedup 1.0000x reference)
"""3D LUT trilinear interpolation (color grading) on 8 Trainium2 NeuronCores.

Slot-space design, data-parallel over batch (one image per core):

  - Table: [128, 32768, 2] fp16 in SBUF. Row 16k+s (s = c*4 + db*2 + dg < 12)
    of core-group k holds, for every cell, the pair (v0, d) = (lut[c, ib+db,
    ig+dg, ir], lut[..., ir+1] - lut[..., ir]).  One ap_gather index per pixel
    fetches all 12 slot pairs of its cell (rows 12-15 are padding).
  - Per tile of N pixels per group (8N total): x is DMA'd planar into rows
    16k+{12,13,14} (r,g,b) of a [128, N] tile; floor/frac use the 1.5*2^23
    magic-number trick; the cell index is assembled on row 16k+12 with two
    stream_shuffles + two scalar_tensor_tensor ops.
  - The gather wants indices wrapped [16k+l, n/16].  That 16 x (N/16)
    transpose rides on the 2-byte indices: one DMA writes the cell row to a
    DRAM scratch laid out [N/16, 128] (columns 16k+l), and one X-bar
    transpose DMA lands it back as the [128, N/16] index tile.
  - Trilinear blend as a lerp tree in slot space: r-lerp uses the (v0, d)
    pair in the free dim (2 ops); g- and b-lerps pair slot rows via
    stream_shuffle row shifts (+1, +2) with 4 ops each.  Fractions are
    replicated to all 16 rows of a group with duplicate-mask stream_shuffles.
  - Channels land on rows 16k+{0,4,8}; one casting DMA writes fp32 planar.
"""

import numpy as np

LUT_DIM = 33
NCELL = 32 * 32 * 32           # 32768
N = 3840                       # pixels per 16-row group per tile (mult of 256)
F = N // 16                    # 240
PIX_TILE = 8 * N               # 30720
NPIX = 1080 * 1920             # per-core pixels = 2073600
NTILES = -(-NPIX // PIX_TILE)  # 68
NP_PAD = NTILES * PIX_TILE

_BINSIZE = np.float32(np.float64(1.000001) / (LUT_DIM - 1))
_SCALE = np.float32(1.0) / _BINSIZE
_MAGIC = float(1.5 * 2.0 ** 23)

_PROG_CACHE = {}


def _build_table(lut):
    """[128, NCELL, 2] fp16: row 16k + (c*4 + db*2 + dg) -> (v0, d) pairs."""
    lut = np.asarray(lut, dtype=np.float32)
    V = np.lib.stride_tricks.sliding_window_view(lut, (2, 2, 2), axis=(1, 2, 3))
    V = V.reshape(3, NCELL, 2, 2, 2)          # (c, cell, db, dg, dr)
    tab16 = np.zeros((16, NCELL, 2), dtype=np.float16)
    for c in range(3):
        for db in range(2):
            for dg in range(2):
                s = c * 4 + db * 2 + dg
                v0 = V[c, :, db, dg, 0]
                v1 = V[c, :, db, dg, 1]
                tab16[s, :, 0] = v0.astype(np.float16)
                tab16[s, :, 1] = (v1 - v0).astype(np.float16)
    return np.tile(tab16, (8, 1, 1))


def _mask(fn):
    return [fn(i) for i in range(32)]


def _build_program(ntiles, n_cores, reps=1):
    import concourse.bacc as bacc
    import concourse.mybir as mybir
    from concourse.tile import TileContext

    fp32 = mybir.dt.float32
    fp16 = mybir.dt.float16
    i16 = mybir.dt.int16
    STT = mybir.AluOpType

    np_pix = ntiles * PIX_TILE
    nc = bacc.Bacc("TRN2", target_bir_lowering=False, debug=False,
                   num_devices=n_cores)
    d_x = nc.dram_tensor("x", [3, np_pix], fp32, kind="ExternalInput")
    d_tab = nc.dram_tensor("tab", [128, NCELL, 2], fp16, kind="ExternalInput")
    d_out = nc.dram_tensor("out", [3, np_pix], fp32, kind="ExternalOutput")
    d_scr = nc.dram_tensor("scr", [F, 128], i16, kind="Internal")

    # stream_shuffle masks (applied per 32-row quadrant; groups of 16)
    m_shp1 = _mask(lambda i: i + 1 if i % 16 in (12, 13) else i)
    m_TR = _mask(lambda i: (i // 16) * 16 + 12)
    m_TG = _mask(lambda i: (i // 16) * 16 + 13)
    m_TB = _mask(lambda i: (i // 16) * 16 + 14)
    m_g = _mask(lambda i: i + 1 if (i % 16 < 12 and i % 2 == 0) else i)
    m_b = _mask(lambda i: i + 2 if (i % 16 < 12 and i % 4 < 2) else i)

    with TileContext(nc) as tc:
        with (tc.tile_pool(name="c", bufs=1) as cp,
              tc.tile_pool(name="w", bufs=1) as wp):
            t_tab = cp.tile([128, NCELL, 2], fp16, tag="tab")
            nc.sync.dma_start(t_tab[:, :, :], d_tab.ap()[:, :, :])
            t_t = wp.tile([128, N], fp32, tag="t")         # xb then t
            t_ff = wp.tile([128, N], fp32, tag="ff")
            t_ff16 = wp.tile([128, N], i16, tag="ff16")
            t_t16 = wp.tile([128, N], fp16, tag="t16")
            t_cell = wp.tile([128, N], i16, tag="cell")
            t_idx = wp.tile([128, F], i16, tag="idx")
            t_V = wp.tile([128, N, 2], fp16, tag="V")
            t_T = wp.tile([128, N], fp16, tag="T")
            # fp16 scratch aliased over the fp32 tiles once those go dead
            ff_b = t_ff[:, :].bitcast(i16)
            t_shg = ff_b[:, 0:N]
            t_shb = ff_b[:, N:2 * N]
            tt_b = t_t[:, :].bitcast(fp16)
            t_L1 = tt_b[:, 0:N]
            t_S = tt_b[:, N:2 * N]
            # order the table load before the pipeline (1-element copy
            # creates tab -> t_t dependency), then zero-init t_t
            nc.vector.tensor_copy(t_t[0:1, 0:1], t_tab[0:1, 0:1, 0])
            nc.vector.memset(t_t[:, :], 0.0)
            for ti0 in range(ntiles * reps):
                ti = ti0 % ntiles
                base = ti * PIX_TILE
                # x[c, k*N+n] -> row 16k+12+c, col n (one DMA per channel;
                # HW DGEs drop the 2nd partition dim of combined APs)
                for c in range(3):
                    nc.gpsimd.dma_start(
                        t_t[:, :].rearrange("(k s) n -> k s n",
                                            k=8)[:, 12 + c, :],
                        d_x.ap()[c, base:base + PIX_TILE].rearrange(
                            "(k n) -> k n", k=8))
                # fi16 = round(x*scale - 0.5) = floor(x*scale); the i16
                # convert rounds to nearest on hardware
                nc.vector.tensor_scalar(t_ff16[:, :], t_t[:, :],
                                        float(_SCALE), -0.5,
                                        STT.mult, STT.add)
                # t16 = frac = x*scale - floor, fp16 out
                nc.vector.scalar_tensor_tensor(
                    t_t16[:, :], t_t[:, :], float(_SCALE), t_ff16[:, :],
                    STT.mult, STT.subtract)
                # align ff_g, ff_b onto the r rows (16k+12); t_ff/t_t fp32
                # buffers are dead now, their bytes hold shg/shb/L1/S
                nc.vector.stream_shuffle(t_shg, t_ff16[:, :], m_shp1)
                nc.vector.stream_shuffle(t_shb, t_shg, m_shp1)
                # cell = (ff_b*32 + ff_g)*32 + ff_r on rows 16k+12 (i16)
                nc.vector.scalar_tensor_tensor(
                    t_cell[:, :], t_shb, 32.0, t_shg,
                    STT.mult, STT.add)
                nc.vector.scalar_tensor_tensor(
                    t_cell[:, :], t_cell[:, :], 32.0, t_ff16[:, :],
                    STT.mult, STT.add)
                # wrap indices: cell row [16k+12, f*16+l] -> scr[f, 16k+l]
                # (one 32B-chunk DMA), then X-bar transpose -> idx[16k+l, f]
                nc.sync.dma_start(
                    d_scr.ap()[:, :].rearrange("f (k l) -> k f l", k=8),
                    t_cell[:, :].rearrange("(k s) (f l) -> k s f l",
                                           k=8, l=16)[:, 12, :, :])
                nc.sync.dma_start_transpose(t_idx[:, :], d_scr.ap()[:, :])
                # gather (v0, d) pairs for all 12 slots of each pixel's cell
                nc.gpsimd.ap_gather(t_V[:, :, :], t_tab[:, :, :],
                                    t_idx[:, :], channels=128,
                                    num_elems=NCELL, d=2, num_idxs=N)
                # r-lerp: L1 = v0 + T_R * d
                nc.vector.stream_shuffle(t_T[:, :], t_t16[:, :], m_TR)
                nc.vector.tensor_mul(t_L1, t_V[:, :, 1], t_T[:, :])
                nc.vector.tensor_add(t_L1, t_L1, t_V[:, :, 0])
                # g-lerp: L2 = L1 + T_G * (shift1(L1) - L1)
                nc.vector.stream_shuffle(t_T[:, :], t_t16[:, :], m_TG)
                nc.vector.stream_shuffle(t_S, t_L1, m_g)
                nc.vector.tensor_sub(t_S, t_S, t_L1)
                nc.vector.tensor_mul(t_S, t_S, t_T[:, :])
                nc.vector.tensor_add(t_L1, t_L1, t_S)
                # b-lerp: L3 = L2 + T_B * (shift2(L2) - L2)
                nc.vector.stream_shuffle(t_T[:, :], t_t16[:, :], m_TB)
                nc.vector.stream_shuffle(t_S, t_L1, m_b)
                nc.vector.tensor_sub(t_S, t_S, t_L1)
                nc.vector.tensor_mul(t_S, t_S, t_T[:, :])
                nc.vector.tensor_add(t_L1, t_L1, t_S)
                # store: rows 16k + 4c (fp16 -> fp32 cast via SWDGE)
                for c in range(3):
                    nc.gpsimd.dma_start(
                        d_out.ap()[c, base:base + PIX_TILE].rearrange(
                            "(k n) -> k n", k=8),
                        t_L1.rearrange("(k s) n -> k s n", k=8)[:, 4 * c, :])

    nc.compile()
    return nc


def _get_program(ntiles, n_cores):
    key = (ntiles, n_cores)
    if key not in _PROG_CACHE:
        _PROG_CACHE[key] = _build_program(ntiles, n_cores)
    return _PROG_CACHE[key]


def kernel(lut, x):
    from concourse import bass_utils

    lut = np.asarray(lut, dtype=np.float32)
    x = np.asarray(x, dtype=np.float32)
    B = x.shape[0]
    tab = _build_table(lut)

    nc = _get_program(NTILES, B)
    in_maps = []
    for b in range(B):
        xb = x[b].reshape(3, -1)
        xpad = np.zeros((3, NP_PAD), dtype=np.float32)
        xpad[:, :xb.shape[1]] = xb
        in_maps.append({"x": xpad, "tab": tab})
    res = bass_utils.run_bass_kernel_spmd(nc, in_maps, core_ids=list(range(B)))
    outs = []
    for b in range(B):
        o = res.results[b]["out"][:, :NPIX]
        outs.append(o.reshape(3, 1080, 1920))
    return np.stack(outs).astype(np.float32)



# revision 5
# speedup vs baseline: 1.6627x; 1.6627x over previous
"""3D LUT trilinear interpolation (color grading) on 8 Trainium2 NeuronCores.

Slot-space design, data-parallel over batch (one image per core):

  - Table: [128, 32768, 2] fp16 in SBUF. Row 16k+s (s = c*4 + db*2 + dg < 12)
    of core-group k holds, for every cell, the pair (v0, d) = (lut[c, ib+db,
    ig+dg, ir], lut[..., ir+1] - lut[..., ir]).  One ap_gather index per pixel
    fetches all 12 slot pairs of its cell (rows 12-15 are padding).
  - Per tile of N pixels per group (8N total): x is DMA'd planar into rows
    16k+{12,13,14} (r,g,b) of a [128, N] tile; floor/frac use the 1.5*2^23
    magic-number trick; the cell index is assembled on row 16k+12 with two
    stream_shuffles + two scalar_tensor_tensor ops.
  - The gather wants indices wrapped [16k+l, n/16].  That 16 x (N/16)
    transpose rides on the 2-byte indices: one DMA writes the cell row to a
    DRAM scratch laid out [N/16, 128] (columns 16k+l), and one X-bar
    transpose DMA lands it back as the [128, N/16] index tile.
  - Trilinear blend as a lerp tree in slot space: r-lerp uses the (v0, d)
    pair in the free dim (2 ops); g- and b-lerps pair slot rows via
    stream_shuffle row shifts (+1, +2) with 4 ops each.  Fractions are
    replicated to all 16 rows of a group with duplicate-mask stream_shuffles.
  - Channels land on rows 16k+{0,4,8}; one casting DMA writes fp32 planar.
"""

import numpy as np

LUT_DIM = 33
NCELL = 32 * 32 * 32           # 32768
N = 3840                       # pixels per 16-row group per tile (mult of 256)
F = N // 16                    # 240
PIX_TILE = 8 * N               # 30720
NPIX = 1080 * 1920             # per-core pixels = 2073600
NTILES = -(-NPIX // PIX_TILE)  # 68
NP_PAD = NTILES * PIX_TILE

_BINSIZE = np.float32(np.float64(1.000001) / (LUT_DIM - 1))
_SCALE = np.float32(1.0) / _BINSIZE
_MAGIC = float(1.5 * 2.0 ** 23)

_PROG_CACHE = {}


def _build_table(lut):
    """[128, NCELL, 2] fp16: row 16k + (c*4 + db*2 + dg) -> (v0, d) pairs."""
    lut = np.asarray(lut, dtype=np.float32)
    V = np.lib.stride_tricks.sliding_window_view(lut, (2, 2, 2), axis=(1, 2, 3))
    V = V.reshape(3, NCELL, 2, 2, 2)          # (c, cell, db, dg, dr)
    tab16 = np.zeros((16, NCELL, 2), dtype=np.float16)
    for c in range(3):
        for db in range(2):
            for dg in range(2):
                s = c * 4 + db * 2 + dg
                v0 = V[c, :, db, dg, 0]
                v1 = V[c, :, db, dg, 1]
                tab16[s, :, 0] = v0.astype(np.float16)
                tab16[s, :, 1] = (v1 - v0).astype(np.float16)
    return np.tile(tab16, (8, 1, 1))


def _mask(fn):
    return [fn(i) for i in range(32)]


def _build_program(ntiles, n_cores, reps=1):
    import concourse.bacc as bacc
    import concourse.mybir as mybir
    from concourse.tile import TileContext

    fp32 = mybir.dt.float32
    fp16 = mybir.dt.float16
    i16 = mybir.dt.int16
    STT = mybir.AluOpType

    np_pix = ntiles * PIX_TILE
    nc = bacc.Bacc("TRN2", target_bir_lowering=False, debug=False,
                   num_devices=n_cores)
    d_x = nc.dram_tensor("x", [3, np_pix], fp32, kind="ExternalInput")
    d_tab = nc.dram_tensor("tab", [128, NCELL, 2], fp16, kind="ExternalInput")
    d_out = nc.dram_tensor("out", [3, np_pix], fp16, kind="ExternalOutput")
    d_scr = nc.dram_tensor("scr", [F, 128], i16, kind="Internal")

    # stream_shuffle masks (applied per 32-row quadrant; groups of 16)
    m_shp1 = _mask(lambda i: i + 1 if i % 16 in (12, 13) else i)
    m_TR = _mask(lambda i: (i // 16) * 16 + 12)
    m_TG = _mask(lambda i: (i // 16) * 16 + 13)
    m_TB = _mask(lambda i: (i // 16) * 16 + 14)
    m_g = _mask(lambda i: i + 1 if (i % 16 < 12 and i % 2 == 0) else i)
    m_b = _mask(lambda i: i + 2 if (i % 16 < 12 and i % 4 < 2) else i)

    with TileContext(nc) as tc:
        with (tc.tile_pool(name="c", bufs=1) as cp,
              tc.tile_pool(name="w", bufs=1) as wp):
            t_tab = cp.tile([128, NCELL, 2], fp16, tag="tab")
            nc.sync.dma_start(t_tab[:, :, :], d_tab.ap()[:, :, :])
            t_t = wp.tile([128, N], fp32, tag="t")         # xb then t
            t_ff = wp.tile([128, N], fp32, tag="ff")
            t_ff16 = wp.tile([128, N], i16, tag="ff16")
            t_t16 = wp.tile([128, N], fp16, tag="t16")
            t_cell = wp.tile([128, N], i16, tag="cell")
            t_idx = wp.tile([128, F], i16, tag="idx")
            t_V = wp.tile([128, N, 2], fp16, tag="V")
            t_T = wp.tile([128, N], fp16, tag="T")
            # fp16 scratch aliased over the fp32 tiles once those go dead
            ff_b = t_ff[:, :].bitcast(i16)
            t_shg = ff_b[:, 0:N]
            t_shb = ff_b[:, N:2 * N]
            tt_b = t_t[:, :].bitcast(fp16)
            t_L1 = tt_b[:, 0:N]
            t_S = tt_b[:, N:2 * N]
            # order the table load before the pipeline (1-element copy
            # creates tab -> t_t dependency), then zero-init t_t
            nc.vector.tensor_copy(t_t[0:1, 0:1], t_tab[0:1, 0:1, 0])
            nc.vector.memset(t_t[:, :], 0.0)
            for ti0 in range(ntiles * reps):
                ti = ti0 % ntiles
                base = ti * PIX_TILE
                # x[c, k*N+n] -> row 16k+12+c, col n (one DMA per channel;
                # HW DGEs drop the 2nd partition dim of combined APs)
                for c in range(3):
                    nc.sync.dma_start(
                        t_t[:, :].rearrange("(k s) n -> k s n",
                                            k=8)[:, 12 + c, :],
                        d_x.ap()[c, base:base + PIX_TILE].rearrange(
                            "(k n) -> k n", k=8))
                # fi16 = round(x*scale - 0.5) = floor(x*scale); the i16
                # convert rounds to nearest on hardware
                nc.vector.tensor_scalar(t_ff16[:, :], t_t[:, :],
                                        float(_SCALE), -0.5,
                                        STT.mult, STT.add)
                # t16 = frac = x*scale - floor, fp16 out
                nc.vector.scalar_tensor_tensor(
                    t_t16[:, :], t_t[:, :], float(_SCALE), t_ff16[:, :],
                    STT.mult, STT.subtract)
                # align ff_g, ff_b onto the r rows (16k+12); t_ff/t_t fp32
                # buffers are dead now, their bytes hold shg/shb/L1/S
                nc.vector.stream_shuffle(t_shg, t_ff16[:, :], m_shp1)
                nc.vector.stream_shuffle(t_shb, t_shg, m_shp1)
                # cell = (ff_b*32 + ff_g)*32 + ff_r on rows 16k+12 (i16)
                nc.vector.scalar_tensor_tensor(
                    t_cell[:, :], t_shb, 32.0, t_shg,
                    STT.mult, STT.add)
                nc.vector.scalar_tensor_tensor(
                    t_cell[:, :], t_cell[:, :], 32.0, t_ff16[:, :],
                    STT.mult, STT.add)
                # wrap indices: cell row [16k+12, f*16+l] -> scr[f, 16k+l]
                # (one 32B-chunk DMA), then X-bar transpose -> idx[16k+l, f]
                nc.sync.dma_start(
                    d_scr.ap()[:, :].rearrange("f (k l) -> k f l", k=8),
                    t_cell[:, :].rearrange("(k s) (f l) -> k s f l",
                                           k=8, l=16)[:, 12, :, :])
                nc.sync.dma_start_transpose(t_idx[:, :], d_scr.ap()[:, :])
                # gather (v0, d) pairs for all 12 slots of each pixel's cell
                nc.gpsimd.ap_gather(t_V[:, :, :], t_tab[:, :, :],
                                    t_idx[:, :], channels=128,
                                    num_elems=NCELL, d=2, num_idxs=N)
                # r-lerp: L1 = v0 + T_R * d
                nc.vector.stream_shuffle(t_T[:, :], t_t16[:, :], m_TR)
                nc.vector.tensor_mul(t_L1, t_V[:, :, 1], t_T[:, :])
                nc.vector.tensor_add(t_L1, t_L1, t_V[:, :, 0])
                # g-lerp: L2 = L1 + T_G * (shift1(L1) - L1)
                nc.vector.stream_shuffle(t_T[:, :], t_t16[:, :], m_TG)
                nc.vector.stream_shuffle(t_S, t_L1, m_g)
                nc.vector.tensor_sub(t_S, t_S, t_L1)
                nc.vector.tensor_mul(t_S, t_S, t_T[:, :])
                nc.vector.tensor_add(t_L1, t_L1, t_S)
                # b-lerp: L3 = L2 + T_B * (shift2(L2) - L2)
                nc.vector.stream_shuffle(t_T[:, :], t_t16[:, :], m_TB)
                nc.vector.stream_shuffle(t_S, t_L1, m_b)
                nc.vector.tensor_sub(t_S, t_S, t_L1)
                nc.vector.tensor_mul(t_S, t_S, t_T[:, :])
                nc.vector.tensor_add(t_L1, t_L1, t_S)
                # store: rows 16k + 4c, fp16 planar via HWDGE (host upcasts)
                for c in range(3):
                    nc.sync.dma_start(
                        d_out.ap()[c, base:base + PIX_TILE].rearrange(
                            "(k n) -> k n", k=8),
                        t_L1.rearrange("(k s) n -> k s n", k=8)[:, 4 * c, :])

    nc.compile()
    return nc


def _get_program(ntiles, n_cores):
    key = (ntiles, n_cores)
    if key not in _PROG_CACHE:
        _PROG_CACHE[key] = _build_program(ntiles, n_cores)
    return _PROG_CACHE[key]


def kernel(lut, x):
    from concourse import bass_utils

    lut = np.asarray(lut, dtype=np.float32)
    x = np.asarray(x, dtype=np.float32)
    B = x.shape[0]
    tab = _build_table(lut)

    nc = _get_program(NTILES, B)
    in_maps = []
    for b in range(B):
        xb = x[b].reshape(3, -1)
        xpad = np.zeros((3, NP_PAD), dtype=np.float32)
        xpad[:, :xb.shape[1]] = xb
        in_maps.append({"x": xpad, "tab": tab})
    res = bass_utils.run_bass_kernel_spmd(nc, in_maps, core_ids=list(range(B)))
    outs = []
    for b in range(B):
        o = res.results[b]["out"][:, :NPIX].astype(np.float32)
        outs.append(o.reshape(3, 1080, 1920))
    return np.stack(outs).astype(np.float32)



# revision 8
# speedup vs baseline: 1.8413x; 1.1074x over previous
"""3D LUT trilinear interpolation on 8 TRN2 NeuronCores — v7.

Serial slot-space pipeline (see kernel.py) at N=6656 (39 tiles).  The whole
i16 scratch chain now cascades through the x tile's bytes, freeing the old
4B/px scratch tile; only a 2B/px T tile remains:

  t_t bytes:  x (fp32) -> [shg | shb/cell] (i16, after frac consumes x)
              -> [L1 | S] (fp16, lerp; L1 write waits out-DMA(t-1))
  t_V bytes:  ff16 (floor, first half) -> gather output (clobbers dead ff16)

All alias hand-offs are ordered by in-order DVE execution plus the framework's
cross-queue WAR semaphores.  Out is fp16 (host upcasts), DMAs on HWDGE.
"""

import numpy as np

LUT_DIM = 33
NCELL = 32 * 32 * 32           # 32768
N = 6656                       # pixels per 16-row group per tile
F = N // 16                    # 416
PIX_TILE = 8 * N               # 53248
NPIX = 1080 * 1920             # per-core pixels = 2073600
NTILES = -(-NPIX // PIX_TILE)  # 39
NP_PAD = NTILES * PIX_TILE

_BINSIZE = np.float32(np.float64(1.000001) / (LUT_DIM - 1))
_SCALE = np.float32(1.0) / _BINSIZE

_PROG_CACHE = {}


def _build_table(lut):
    """[128, NCELL, 2] fp16: row 16k + (c*4 + db*2 + dg) -> (v0, d) pairs."""
    lut = np.asarray(lut, dtype=np.float32)
    V = np.lib.stride_tricks.sliding_window_view(lut, (2, 2, 2), axis=(1, 2, 3))
    V = V.reshape(3, NCELL, 2, 2, 2)          # (c, cell, db, dg, dr)
    tab16 = np.zeros((16, NCELL, 2), dtype=np.float16)
    for c in range(3):
        for db in range(2):
            for dg in range(2):
                s = c * 4 + db * 2 + dg
                v0 = V[c, :, db, dg, 0]
                v1 = V[c, :, db, dg, 1]
                tab16[s, :, 0] = v0.astype(np.float16)
                tab16[s, :, 1] = (v1 - v0).astype(np.float16)
    return np.tile(tab16, (8, 1, 1))


def _mask(fn):
    return [fn(i) for i in range(32)]


def _build_program(ntiles, n_cores, reps=1):
    import concourse.bacc as bacc
    import concourse.mybir as mybir
    from concourse.tile import TileContext

    fp32 = mybir.dt.float32
    fp16 = mybir.dt.float16
    i16 = mybir.dt.int16
    STT = mybir.AluOpType

    np_pix = ntiles * PIX_TILE
    nc = bacc.Bacc("TRN2", target_bir_lowering=False, debug=False,
                   num_devices=n_cores)
    d_x = nc.dram_tensor("x", [3, np_pix], fp32, kind="ExternalInput")
    d_tab = nc.dram_tensor("tab", [128, NCELL, 2], fp16, kind="ExternalInput")
    d_out = nc.dram_tensor("out", [3, np_pix], fp16, kind="ExternalOutput")
    d_scr = nc.dram_tensor("scr", [F, 128], i16, kind="Internal")

    # stream_shuffle masks (applied per 32-row quadrant; groups of 16)
    m_shp1 = _mask(lambda i: i + 1 if i % 16 in (12, 13) else i)
    m_TR = _mask(lambda i: (i // 16) * 16 + 12)
    m_TG = _mask(lambda i: (i // 16) * 16 + 13)
    m_TB = _mask(lambda i: (i // 16) * 16 + 14)
    m_g = _mask(lambda i: i + 1 if (i % 16 < 12 and i % 2 == 0) else i)
    m_b = _mask(lambda i: i + 2 if (i % 16 < 12 and i % 4 < 2) else i)

    with TileContext(nc) as tc:
        with (tc.tile_pool(name="c", bufs=1) as cp,
              tc.tile_pool(name="w", bufs=1) as wp):
            t_tab = cp.tile([128, NCELL, 2], fp16, tag="tab")
            nc.sync.dma_start(t_tab[:, :, :], d_tab.ap()[:, :, :])
            t_t = wp.tile([128, N], fp32, tag="t")    # x -> shg|shb -> L1|S
            t_t16 = wp.tile([128, N], fp16, tag="t16")
            t_T = wp.tile([128, N], fp16, tag="T")
            t_idx = wp.tile([128, F], i16, tag="idx")
            t_V = wp.tile([128, N, 2], fp16, tag="V")
            t_ff16 = t_V[:, :, :].rearrange(
                "p n two -> p (n two)").bitcast(i16)[:, 0:N]
            tt_i = t_t[:, :].bitcast(i16)
            t_shg = tt_i[:, 0:N]
            t_shb = tt_i[:, N:2 * N]
            t_cell = t_shb                      # in-place cell assembly
            tt_b = t_t[:, :].bitcast(fp16)
            t_L1 = tt_b[:, 0:N]
            t_S = tt_b[:, N:2 * N]
            # order the table load before the pipeline, then zero-init t_t
            nc.vector.tensor_copy(t_t[0:1, 0:1], t_tab[0:1, 0:1, 0])
            nc.vector.memset(t_t[:, :], 0.0)
            nc.vector.memset(t_V[:, :, :], 0.0)
            for ti0 in range(ntiles * reps):
                ti = ti0 % ntiles
                base = ti * PIX_TILE
                # x[c, k*N+n] -> row 16k+12+c, col n
                for c in range(3):
                    nc.sync.dma_start(
                        t_t[:, :].rearrange("(k s) n -> k s n",
                                            k=8)[:, 12 + c, :],
                        d_x.ap()[c, base:base + PIX_TILE].rearrange(
                            "(k n) -> k n", k=8))
                # fi16 = round(x*scale - 0.5) = floor(x*scale); lives in V
                nc.vector.tensor_scalar(t_ff16, t_t[:, :],
                                        float(_SCALE), -0.5,
                                        STT.mult, STT.add)
                # t16 = frac = x*scale - floor; last reader of x
                nc.vector.scalar_tensor_tensor(
                    t_t16[:, :], t_t[:, :], float(_SCALE), t_ff16,
                    STT.mult, STT.subtract)
                # align ff_g, ff_b onto the r rows; scratch reuses x bytes
                nc.vector.stream_shuffle(t_shg, t_ff16, m_shp1)
                nc.vector.stream_shuffle(t_shb, t_shg, m_shp1)
                # cell = (ff_b*32 + ff_g)*32 + ff_r, in place on shb
                nc.vector.scalar_tensor_tensor(
                    t_cell, t_shb, 32.0, t_shg,
                    STT.mult, STT.add)
                nc.vector.scalar_tensor_tensor(
                    t_cell, t_cell, 32.0, t_ff16,
                    STT.mult, STT.add)
                # wrap indices via DRAM bounce + X-bar transpose
                nc.sync.dma_start(
                    d_scr.ap()[:, :].rearrange("f (k l) -> k f l", k=8),
                    t_cell.rearrange("(k s) (f l) -> k s f l",
                                     k=8, l=16)[:, 12, :, :])
                nc.sync.dma_start_transpose(t_idx[:, :], d_scr.ap()[:, :])
                # gather (v0, d) pairs; output clobbers the dead ff16
                nc.gpsimd.ap_gather(t_V[:, :, :], t_tab[:, :, :],
                                    t_idx[:, :], channels=128,
                                    num_elems=NCELL, d=2, num_idxs=N)
                # r-lerp: L1 = v0 + T_R * d  (L1 overwrites dead shg bytes)
                nc.vector.stream_shuffle(t_T[:, :], t_t16[:, :], m_TR)
                nc.vector.tensor_mul(t_L1, t_V[:, :, 1], t_T[:, :])
                nc.vector.tensor_add(t_L1, t_L1, t_V[:, :, 0])
                # g-lerp: L2 = L1 + T_G * (shift1(L1) - L1)
                nc.vector.stream_shuffle(t_T[:, :], t_t16[:, :], m_TG)
                nc.vector.stream_shuffle(t_S, t_L1, m_g)
                nc.vector.tensor_sub(t_S, t_S, t_L1)
                nc.vector.tensor_mul(t_S, t_S, t_T[:, :])
                nc.vector.tensor_add(t_L1, t_L1, t_S)
                # b-lerp: L3 = L2 + T_B * (shift2(L2) - L2)
                nc.vector.stream_shuffle(t_T[:, :], t_t16[:, :], m_TB)
                nc.vector.stream_shuffle(t_S, t_L1, m_b)
                nc.vector.tensor_sub(t_S, t_S, t_L1)
                nc.vector.tensor_mul(t_S, t_S, t_T[:, :])
                nc.vector.tensor_add(t_L1, t_L1, t_S)
                # store: rows 16k + 4c, fp16 planar via HWDGE (host upcasts)
                for c in range(3):
                    nc.sync.dma_start(
                        d_out.ap()[c, base:base + PIX_TILE].rearrange(
                            "(k n) -> k n", k=8),
                        t_L1.rearrange("(k s) n -> k s n", k=8)[:, 4 * c, :])

    nc.compile()
    return nc


def _get_program(ntiles, n_cores):
    key = (ntiles, n_cores)
    if key not in _PROG_CACHE:
        _PROG_CACHE[key] = _build_program(ntiles, n_cores)
    return _PROG_CACHE[key]


def kernel(lut, x):
    from concourse import bass_utils

    lut = np.asarray(lut, dtype=np.float32)
    x = np.asarray(x, dtype=np.float32)
    B = x.shape[0]
    tab = _build_table(lut)

    nc = _get_program(NTILES, B)
    in_maps = []
    for b in range(B):
        xb = x[b].reshape(3, -1)
        xpad = np.zeros((3, NP_PAD), dtype=np.float32)
        xpad[:, :xb.shape[1]] = xb
        in_maps.append({"x": xpad, "tab": tab})
    res = bass_utils.run_bass_kernel_spmd(nc, in_maps, core_ids=list(range(B)))
    outs = []
    for b in range(B):
        o = res.results[b]["out"][:, :NPIX].astype(np.float32)
        outs.append(o.reshape(3, 1080, 1920))
    return np.stack(outs).astype(np.float32)
